# revision 4
# baseline (speedup 1.0000x reference)
"""MoE router kernel (LayerNorm -> Linear -> ReLU -> Linear -> softmax -> top-2
+ aux loss) for one TRN2 chip (8 NeuronCores, data-parallel over tokens).

Numerics: the big matmul runs as a 3-term bf16 split
    x @ W ~= x_hi@W_hi + x_lo@W_hi + x_hi@W_lo     (hi/lo bf16)
which is f32-accurate (logit err ~1e-6) so the top-2 indices match the f32
reference exactly.  The small router matmul and everything else is fp32.
LayerNorm is folded into a post-matmul affine:
    LN(x) @ W1g = r*(x @ W1g) + (-r*mu)*colsum(W1g)        (r, mu per token)
so raw x^T feeds the matmul and no normalized copy is materialized.
"""

import os
import sys
from contextlib import ExitStack

import numpy as np
import ml_dtypes

import concourse.bass as bass
import concourse.bacc as bacc
import concourse.mybir as mybir
import concourse.tile as tile
from concourse.bass_utils import run_bass_kernel_spmd
from concourse.masks import make_identity

# ---- problem constants (hardcoded per spec) ----
B, S, H, E, TOPK = 4, 4096, 2048, 64, 2
LN_EPS = 1e-5
AUX_EPS = 1e-9
N_CORES = 8
TOK = B * S                      # 16384
TPC = TOK // N_CORES             # tokens per core: 2048
P = 128
CHUNK = 512                      # tokens per chunk
N_CHUNKS = TPC // CHUNK          # 4
TPCH = CHUNK // P                # token tiles per chunk: 4
N_TILES = TPC // P               # 16
KB = H // P                      # 16 contraction slices
MB = H // P                      # 16 output (H_out) slices

FP32 = mybir.dt.float32
BF16 = mybir.dt.bfloat16
I32 = mybir.dt.int32
U32 = mybir.dt.uint32
AF = mybir.ActivationFunctionType
OP = mybir.AluOpType


def build_program():
    nc = bacc.Bacc(None, target_bir_lowering=False)

    x_d = nc.dram_tensor("x", [TPC, H], FP32, kind="ExternalInput")
    # weight layouts are host-pre-tiled so every DMA is per-partition contiguous
    whi_d = nc.dram_tensor("w_hi", [MB * P, KB * P], BF16, kind="ExternalInput")
    wlo_d = nc.dram_tensor("w_lo", [MB * P, KB * P], BF16, kind="ExternalInput")
    w2_d = nc.dram_tensor("w2", [P, KB * E], FP32, kind="ExternalInput")
    csum_d = nc.dram_tensor("csum", [P, MB], FP32, kind="ExternalInput")
    b1g_d = nc.dram_tensor("b1g", [P, MB], FP32, kind="ExternalInput")
    b2_d = nc.dram_tensor("b2", [1, E], FP32, kind="ExternalInput")

    oidx_d = nc.dram_tensor("out_idx", [P, N_TILES * TOPK], I32, kind="ExternalOutput")
    op_d = nc.dram_tensor("out_p", [P, N_TILES * TOPK], FP32, kind="ExternalOutput")
    oacc_d = nc.dram_tensor("out_acc", [P, E], FP32, kind="ExternalOutput")

    with tile.TileContext(nc) as tc, ExitStack() as ctx:
        consts = ctx.enter_context(tc.tile_pool(name="consts", bufs=1))
        xnat_p = ctx.enter_context(tc.tile_pool(name="xnat", bufs=6))
        stat_p = ctx.enter_context(tc.tile_pool(name="stats", bufs=6))
        xt_p = ctx.enter_context(tc.tile_pool(name="xt", bufs=1))
        h_p = ctx.enter_context(tc.tile_pool(name="h", bufs=1))
        w_p = ctx.enter_context(tc.tile_pool(name="w", bufs=3))
        ep_p = ctx.enter_context(tc.tile_pool(name="ep", bufs=2))
        rs_p = ctx.enter_context(tc.tile_pool(name="rs", bufs=2))
        small_p = ctx.enter_context(tc.tile_pool(name="small", bufs=2))

        ps_T = ctx.enter_context(tc.tile_pool(name="psT", bufs=2, space="PSUM"))
        ps_y = ctx.enter_context(tc.tile_pool(name="psy", bufs=2, space="PSUM"))
        ps_L = ctx.enter_context(tc.tile_pool(name="psL", bufs=1, space="PSUM"))
        ps_LT = ctx.enter_context(tc.tile_pool(name="psLT", bufs=1, space="PSUM"))
        ps_row = ctx.enter_context(tc.tile_pool(name="psrow", bufs=1, space="PSUM"))
        ps_R = ctx.enter_context(tc.tile_pool(name="psR", bufs=1, space="PSUM"))

        # ---- constants ----
        ident = consts.tile([P, P], FP32)
        make_identity(nc, ident)
        ones_row = consts.tile([1, CHUNK], FP32)
        nc.vector.memset(ones_row, 1.0)
        eps_t = consts.tile([P, 1], FP32)
        nc.vector.memset(eps_t, LN_EPS)

        w2_sb = consts.tile([P, KB * E], FP32)
        nc.sync.dma_start(w2_sb, w2_d[:])
        b2_sb = consts.tile([1, E], FP32)
        nc.sync.dma_start(b2_sb, b2_d[:])
        csum_sb = consts.tile([P, MB], FP32)
        nc.sync.dma_start(csum_sb, csum_d[:])
        b1g_sb = consts.tile([P, MB], FP32)
        nc.sync.dma_start(b1g_sb, b1g_d[:])

        logits_all = consts.tile([P, N_TILES, E], FP32)
        m8_all = consts.tile([P, N_TILES, 8], FP32)
        idx_all = consts.tile([P, N_TILES, 8], U32)

        for c in range(N_CHUNKS):
            # ---- load x tiles (natural layout) ----
            xts = []
            for i in range(TPCH):
                xt = xnat_p.tile([P, H], FP32, tag="xnat")
                t0 = (c * TPCH + i) * P
                nc.sync.dma_start(xt, x_d[t0 : t0 + P, :])
                xts.append(xt)

            # ---- LayerNorm stats per token tile ----
            rcols, nmucols = [], []
            for i, xt in enumerate(xts):
                st = stat_p.tile([P, 4, 6], FP32, tag="bnst")
                xv = xt[:].rearrange("p (a b) -> p a b", a=4)
                for j in range(4):
                    nc.vector.bn_stats(st[:, j, :], xv[:, j, :])
                mv = stat_p.tile([P, 2], FP32, tag="mv")
                nc.vector.bn_aggr(mv, st[:])
                rc = stat_p.tile([P, 1], FP32, tag="rc")
                nm = stat_p.tile([P, 1], FP32, tag="nm")
                # rc = 1/sqrt(var+eps); nm = -mu
                nc.scalar.activation(rc, mv[:, 1:2], AF.Sqrt, bias=eps_t[:], scale=1.0)
                nc.vector.reciprocal(rc, rc)
                nc.vector.tensor_scalar_mul(nm, mv[:, 0:1], -1.0)
                rcols.append(rc)
                nmucols.append(nm)

            # ---- per-token scalars broadcast to [P, CHUNK] rows ----
            # transpose columns -> psum row [1, CHUNK], copy to sbuf,
            # then ones-outer-product matmul broadcasts across partitions
            r_bc = rs_p.tile([P, CHUNK], FP32, tag="rbc")
            m_bc = rs_p.tile([P, CHUNK], FP32, tag="mbc")
            for cols, bc in ((rcols, r_bc), (nmucols, m_bc)):
                prow = ps_row.tile([1, CHUNK], FP32, tag="prow")
                for i in range(TPCH):
                    nc.tensor.transpose(prow[:, i * P : (i + 1) * P], cols[i], ident[:])
                srow = small_p.tile([1, CHUNK], FP32, tag="srow")
                nc.scalar.copy(srow, prow[:])
                pR = ps_R.tile([P, CHUNK], FP32, tag="pR")
                nc.tensor.matmul(pR[:], ones_row[:1, :P], srow[:], start=True, stop=True)
                nc.scalar.copy(bc, pR[:])

            # ---- transpose x -> x^T, split into bf16 hi/lo ----
            xhi = xt_p.tile([P, KB, CHUNK], BF16, tag="xhi")
            xlo = xt_p.tile([P, KB, CHUNK], BF16, tag="xlo")
            for k in range(KB):
                pT = ps_T.tile([P, CHUNK], FP32, tag="pT")
                for i in range(TPCH):
                    nc.tensor.transpose(
                        pT[:, i * P : (i + 1) * P], xts[i][:, k * P : (k + 1) * P], ident[:]
                    )
                nc.scalar.copy(xhi[:, k, :], pT[:])
                nc.vector.tensor_sub(xlo[:, k, :], pT[:], xhi[:, k, :])

            # ---- matmul1 (3-term bf16 split) + fused LN epilogue ----
            hs = h_p.tile([P, MB, CHUNK], FP32, tag="h")
            for m in range(MB):
                wh = w_p.tile([P, KB * P], BF16, tag="wh")
                wl = w_p.tile([P, KB * P], BF16, tag="wl")
                nc.sync.dma_start(wh, whi_d[m * P : (m + 1) * P, :])
                nc.sync.dma_start(wl, wlo_d[m * P : (m + 1) * P, :])
                py = ps_y.tile([P, CHUNK], FP32, tag="py")
                for k in range(KB):
                    whk = wh[:, k * P : (k + 1) * P]
                    wlk = wl[:, k * P : (k + 1) * P]
                    nc.tensor.matmul(py[:], whk, xhi[:, k, :], start=(k == 0), stop=False)
                    nc.tensor.matmul(py[:], whk, xlo[:, k, :], start=False, stop=False)
                    nc.tensor.matmul(py[:], wlk, xhi[:, k, :], start=False, stop=(k == KB - 1))
                # h = relu(r*y + (-r*mu)*csum_m + b1g_m)
                #   = relu( r_bc * (m_bc * csum_m + y) + b1g_m )
                t1 = ep_p.tile([P, CHUNK], FP32, tag="ept1")
                nc.vector.scalar_tensor_tensor(
                    t1, in0=m_bc[:], scalar=csum_sb[:, m : m + 1], in1=py[:],
                    op0=OP.mult, op1=OP.add,
                )
                t2 = ep_p.tile([P, CHUNK], FP32, tag="ept2")
                nc.vector.tensor_mul(t2, r_bc[:], t1[:])
                nc.scalar.activation(
                    hs[:, m, :], t2[:], AF.Relu, bias=b1g_sb[:, m : m + 1], scale=1.0
                )

            # ---- matmul2 (fp32) + bias via K=1 ones row ----
            pL = ps_L.tile([E, CHUNK], FP32, tag="pL")
            for k in range(KB):
                nc.tensor.matmul(
                    pL[:], w2_sb[:, k * E : (k + 1) * E], hs[:, k, :],
                    start=(k == 0), stop=False,
                )
            nc.tensor.matmul(pL[:], b2_sb[:], ones_row[:], start=False, stop=True)

            # ---- logits -> [tokens, E] tiles + top-8 ----
            lsb = small_p.tile([E, CHUNK], FP32, tag="lsb")
            nc.scalar.copy(lsb, pL[:])
            for i in range(TPCH):
                tg = c * TPCH + i
                pLT = ps_LT.tile([P, E], FP32, tag="pLT")
                nc.tensor.transpose(pLT[:], lsb[:, i * P : (i + 1) * P], ident[:E, :E])
                nc.scalar.copy(logits_all[:, tg, :], pLT[:])
                nc.vector.max(m8_all[:, tg, :], logits_all[:, tg, :])
                nc.vector.max_index(idx_all[:, tg, :], m8_all[:, tg, :], logits_all[:, tg, :])

        # ---- batched softmax (for aux partial sums) ----
        probs = consts.tile([P, N_TILES, E], FP32)
        nc.scalar.activation(probs[:], logits_all[:], AF.Exp)
        ssum = consts.tile([P, N_TILES], FP32)
        nc.vector.reduce_sum(ssum, probs[:], axis=mybir.AxisListType.X)
        nc.vector.reciprocal(ssum, ssum)
        sb = ssum[:].to_broadcast([P, N_TILES, E])
        nc.vector.tensor_mul(probs[:], probs[:], sb)
        acc = consts.tile([P, E], FP32)
        nc.vector.reduce_sum(acc, probs[:].rearrange("p t e -> p e t"),
                             axis=mybir.AxisListType.X)

        # ---- renormalized top-2 probs: p1 = 1/(1+exp(l2-l1)), p2 = 1-p1 ----
        d16 = consts.tile([P, N_TILES], FP32)
        nc.vector.tensor_sub(d16, m8_all[:, :, 1], m8_all[:, :, 0])
        e16 = consts.tile([P, N_TILES], FP32)
        nc.scalar.activation(e16, d16[:], AF.Exp)
        den = consts.tile([P, N_TILES], FP32)
        nc.vector.tensor_scalar_add(den, e16[:], 1.0)
        p1 = consts.tile([P, N_TILES], FP32)
        nc.vector.reciprocal(p1, den[:])
        p2 = consts.tile([P, N_TILES], FP32)
        nc.vector.tensor_mul(p2, e16[:], p1[:])

        outp = consts.tile([P, N_TILES, TOPK], FP32)
        nc.vector.tensor_copy(outp[:, :, 0], p1[:])
        nc.vector.tensor_copy(outp[:, :, 1], p2[:])
        outi = consts.tile([P, N_TILES, TOPK], I32)
        nc.vector.tensor_copy(outi[:], idx_all[:, :, 0:TOPK])

        nc.sync.dma_start(oidx_d[:].rearrange("p (t k) -> p t k", k=TOPK), outi[:])
        nc.sync.dma_start(op_d[:].rearrange("p (t k) -> p t k", k=TOPK), outp[:])
        nc.sync.dma_start(oacc_d[:], acc[:])

    nc.finalize()
    return nc


_CACHE = {}


def _get_program():
    if "nc" not in _CACHE:
        _CACHE["nc"] = build_program()
    return _CACHE["nc"]


def _prep_inputs(x, W1, b1, W2, b2, gamma, beta):
    bf16 = ml_dtypes.bfloat16
    xf = np.ascontiguousarray(np.asarray(x, dtype=np.float32).reshape(TOK, H))
    W1 = np.asarray(W1, np.float32)
    gamma = np.asarray(gamma, np.float32)
    beta = np.asarray(beta, np.float32)
    b1 = np.asarray(b1, np.float32)
    W1g = (gamma[:, None] * W1).astype(np.float32)
    b1g = (b1 + beta @ W1).astype(np.float32)
    W_hi = W1g.astype(bf16)
    W_hi32 = W_hi.astype(np.float32)
    W_lo = (W1g - W_hi32).astype(bf16)
    csum = (W_hi32 + W_lo.astype(np.float32)).sum(axis=0, dtype=np.float32)

    def tile_w(w):  # [H_in, H_out] -> [MB*P, KB*P] with [m*P+p, k*P+j] = w[k*P+j_in...]
        return np.ascontiguousarray(
            w.reshape(KB, P, MB, P).transpose(2, 1, 0, 3).reshape(MB * P, KB * P)
        )

    whi_t = tile_w(W_hi)
    wlo_t = tile_w(W_lo)
    w2_t = np.ascontiguousarray(
        np.asarray(W2, np.float32).reshape(KB, P, E).transpose(1, 0, 2).reshape(P, KB * E)
    )
    csum_t = np.ascontiguousarray(csum.reshape(MB, P).T)
    b1g_t = np.ascontiguousarray(b1g.reshape(MB, P).T)
    b2_t = np.ascontiguousarray(np.asarray(b2, np.float32).reshape(1, E))

    in_maps = []
    for c in range(N_CORES):
        in_maps.append(
            {
                "x": xf[c * TPC : (c + 1) * TPC],
                "w_hi": whi_t,
                "w_lo": wlo_t,
                "w2": w2_t,
                "csum": csum_t,
                "b1g": b1g_t,
                "b2": b2_t,
            }
        )
    return in_maps


def _assemble(results):
    idx_parts, p_parts, accs = [], [], []
    for c in range(N_CORES):
        r = results[c]
        idx_parts.append(
            np.asarray(r["out_idx"]).reshape(P, N_TILES, TOPK).transpose(1, 0, 2).reshape(TPC, TOPK)
        )
        p_parts.append(
            np.asarray(r["out_p"]).reshape(P, N_TILES, TOPK).transpose(1, 0, 2).reshape(TPC, TOPK)
        )
        accs.append(np.asarray(r["out_acc"], dtype=np.float32))
    idx = np.concatenate(idx_parts, 0).reshape(B, S, TOPK).astype(np.int32)
    p = np.concatenate(p_parts, 0).reshape(B, S, TOPK).astype(np.float32)
    p_expert = (np.stack(accs).sum(axis=(0, 1)) / np.float32(TOK)).astype(np.float32)
    aux = np.float32(np.sum(p_expert * np.log(p_expert * np.float32(E) + np.float32(AUX_EPS))))
    return idx, p, aux


def kernel(x, W1, b1, W2, b2, gamma, beta, _result_cache={}):
    nc = _get_program()
    in_maps = _prep_inputs(x, W1, b1, W2, b2, gamma, beta)
    res = run_bass_kernel_spmd(nc, in_maps, core_ids=list(range(N_CORES)))
    _result_cache["last"] = res
    idx, p, aux = _assemble(res.results)
    return idx, p, aux
